# revision 15
# baseline (speedup 1.0000x reference)
"""Bayesian routing strategy (MoE routing) Bass kernel for 8 TRN2 NeuronCores.

Data-parallel over tokens: each core handles B/8 = 2048 tokens with
replicated router params (W1/b1/W2/b2). Math per core:
    h  = relu(x @ W1 + b1) * 1/(1-p)                    [hT layout: 128h x b]
    hm_s = h * (mask1_u[s] >= p)                        (5 samples)
    logits_s = (hm_s @ W2 + b2) * (mask2_u[s] >= p)/(1-p)   [128b x 5 x 64e]
    probs = softmax(mean_s logits); all_probs = softmax(logits_s)
    unc = mean_e std_s(all_probs, ddof=1)
    top4 of probs via DVE max8/max_index; extra 2 gated by unc > 0.3.

All matmuls fp32 (fp32r measured at ~1e-4 — too lossy for stable top-k).
x must be transposed (contraction dim on partitions): done on PE
(fp32 transpose = 2 cyc/row) interleaved with the fp32 matmuls.
"""

import sys

if "/opt/trn_rl_repo" not in sys.path:
    sys.path.insert(0, "/opt/trn_rl_repo")

import numpy as np

import concourse.bacc as bacc
import concourse.mybir as mybir
import concourse.tile as tile
from concourse.masks import make_identity
from concourse.bass_utils import run_bass_kernel_spmd

F32 = mybir.dt.float32
I32 = mybir.dt.int32
U32 = mybir.dt.uint32
Alu = mybir.AluOpType
Act = mybir.ActivationFunctionType
AxX = mybir.AxisListType.X

P_DROP = 0.3
SCALE = 1.0 / (1.0 - P_DROP)
THR = 0.3
S = 5
D = 4096
H = 128
E = 64
N_CORES = 8
B_FULL = 16384


def build(b_loc=B_FULL // N_CORES):
    """Build the per-core SPMD program. b_loc = tokens per core (mult of 512)."""
    nblk = b_loc // 512
    nt = b_loc // 128  # b-tiles per core
    KCH = D // 128  # 32 k-chunks
    nc = bacc.Bacc(None, target_bir_lowering=False, debug=False)

    x_d = nc.dram_tensor("xs", [b_loc, D], F32, kind="ExternalInput")
    m1_d = nc.dram_tensor("m1s", [S, b_loc, H], F32, kind="ExternalInput")
    m2_d = nc.dram_tensor("m2s", [S, b_loc, E], F32, kind="ExternalInput")
    w1_d = nc.dram_tensor("w1", [D, H], F32, kind="ExternalInput")
    b1_d = nc.dram_tensor("b1", [H], F32, kind="ExternalInput")
    w2_d = nc.dram_tensor("w2", [H, E], F32, kind="ExternalInput")
    b2_d = nc.dram_tensor("b2", [E], F32, kind="ExternalInput")

    ip_d = nc.dram_tensor("idx_pack", [128, nt, 4], I32, kind="ExternalOutput")
    pp_d = nc.dram_tensor("prob_pack", [128, nt, 4], F32, kind="ExternalOutput")
    up_d = nc.dram_tensor("unc_pack", [128, nt], F32, kind="ExternalOutput")

    with tile.TileContext(nc) as tc:
        with (
            tc.tile_pool(name="const", bufs=1) as cpool,
            tc.tile_pool(name="outp", bufs=1) as opool,
            tc.tile_pool(name="xn", bufs=3) as xpool,
            tc.tile_pool(name="xt", bufs=4) as xtpool,
            tc.tile_pool(name="hsb", bufs=2) as hpool,
            tc.tile_pool(name="m1n", bufs=2) as m1pool,
            tc.tile_pool(name="m1t", bufs=2) as m1tpool,
            tc.tile_pool(name="hm", bufs=2) as hmpool,
            tc.tile_pool(name="m2", bufs=2) as m2pool,
            tc.tile_pool(name="eb", bufs=2) as ebig,  # double-buffered epilogue
            tc.tile_pool(name="ep", bufs=1) as ep,  # single-block epilogue scratch
            tc.tile_pool(name="eo", bufs=2) as eout,  # per-tile topk outputs
            tc.tile_pool(name="tps", bufs=3, space="PSUM") as tps,
            tc.tile_pool(name="hps", bufs=1, space="PSUM") as hps,
            tc.tile_pool(name="lgps", bufs=1, space="PSUM") as lgps,
        ):
            ident = cpool.tile([128, 128], F32, tag="ident")
            make_identity(nc, ident[:])

            w1_sb = cpool.tile([128, KCH, H], F32, tag="w1")
            nc.gpsimd.dma_start(w1_sb[:], w1_d[:].rearrange("(k p) h -> p k h", p=128))
            w2_sb = cpool.tile([128, E], F32, tag="w2")
            nc.scalar.dma_start(w2_sb[:], w2_d[:])
            b1_sb = cpool.tile([128, 1], F32, tag="b1")
            nc.scalar.dma_start(b1_sb[:], b1_d[:].rearrange("(p one) -> p one", one=1))
            b1s = cpool.tile([128, 1], F32, tag="b1s")
            nc.vector.tensor_scalar_mul(b1s[:], b1_sb[:], SCALE)
            b2_row = cpool.tile([1, E], F32, tag="b2row")
            nc.scalar.dma_start(b2_row[:], b2_d[:].rearrange("(one e) -> one e", one=1))
            b2_bc = cpool.tile([128, E], F32, tag="b2bc")
            nc.gpsimd.partition_broadcast(b2_bc[:], b2_row[:])
            b2s_bc = cpool.tile([128, E], F32, tag="b2sbc")
            nc.vector.tensor_scalar_mul(b2s_bc[:], b2_bc[:], SCALE)

            ip = opool.tile([128, nt, 4], I32, tag="ip")
            pp = opool.tile([128, nt, 4], F32, tag="pp")
            up = opool.tile([128, nt], F32, tag="up")

            def make_lgb_part(blk, j, hm, lgb4):
                def emit():
                    lg_ps = lgps.tile([128, S, E], F32, tag="lgps")
                    for s in range(S):
                        nc.tensor.matmul(
                            lg_ps[:, s, :],
                            hm[s][:, j * 128 : (j + 1) * 128],
                            w2_sb[:],
                            start=True,
                            stop=True,
                        )
                    # lgb = logits*scale + b2*scale  (still to be masked)
                    nc.vector.scalar_tensor_tensor(
                        lgb4[:, j, :, :],
                        lg_ps[:],
                        SCALE,
                        b2s_bc[:][:, None, :].to_broadcast([128, S, E]),
                        op0=Alu.mult,
                        op1=Alu.add,
                    )

                return emit

            def make_batch_stages(blk, lgb4):
                t0 = blk * 4
                BSE = [128, 4, S, E]
                st = {}

                def stage_a():
                    m2u = m2pool.tile(BSE, F32, tag="m2u")
                    for j in range(4):
                        t = t0 + j
                        nc.scalar.dma_start(
                            m2u[:, j, :, :],
                            m2_d[:, t * 128 : (t + 1) * 128, :].rearrange(
                                "s p e -> p s e"
                            ),
                        )
                    lgd = ebig.tile(BSE, F32, tag="lgd")
                    st["lgd"] = lgd
                    nc.vector.scalar_tensor_tensor(
                        lgd[:].rearrange("p j s e -> p (j s) e"),
                        m2u[:].rearrange("p j s e -> p (j s) e"),
                        P_DROP,
                        lgb4[:].rearrange("p j s e -> p (j s) e"),
                        op0=Alu.is_ge,
                        op1=Alu.mult,
                    )
                    # mean softmax over experts of (sum_s lgd)/S
                    ml = ep.tile([128, 4, E], F32, tag="ml")
                    st["ml"] = ml
                    nc.vector.tensor_reduce(
                        ml[:],
                        lgd[:].rearrange("p j s e -> p j e s"),
                        axis=AxX,
                        op=Alu.add,
                    )
                    rmax = ep.tile([128, 4, 1], F32, tag="rmax")
                    nc.vector.tensor_reduce(rmax[:], ml[:], axis=AxX, op=Alu.max)
                    nc.vector.tensor_sub(
                        ml[:], ml[:], rmax[:].to_broadcast([128, 4, E])
                    )

                def stage_b():
                    lgd, ml = st["lgd"], st["ml"]
                    pex = ep.tile([128, 4, E], F32, tag="pex")
                    nc.scalar.activation(pex[:], ml[:], Act.Exp, scale=1.0 / S)
                    psm = ep.tile([128, 4, 1], F32, tag="psm")
                    nc.vector.tensor_reduce(psm[:], pex[:], axis=AxX, op=Alu.add)
                    prc = ep.tile([128, 4, 1], F32, tag="prc")
                    nc.vector.reciprocal(prc[:], psm[:])
                    probs = ep.tile([128, 4, E], F32, tag="probs")
                    st["probs"] = probs
                    nc.vector.tensor_mul(
                        probs[:], pex[:], prc[:].to_broadcast([128, 4, E])
                    )

                    # per-sample softmax (in place over lgd)
                    rmx = ep.tile([128, 4, S, 1], F32, tag="rmx")
                    nc.vector.tensor_reduce(rmx[:], lgd[:], axis=AxX, op=Alu.max)
                    nc.vector.tensor_sub(lgd[:], lgd[:], rmx[:].to_broadcast(BSE))

                def stage_c():
                    lgd = st["lgd"]
                    aex = ebig.tile(BSE, F32, tag="aex")
                    st["aex"] = aex
                    nc.scalar.activation(aex[:], lgd[:], Act.Exp)
                    asm = ep.tile([128, 4, S, 1], F32, tag="asm")
                    nc.vector.tensor_reduce(asm[:], aex[:], axis=AxX, op=Alu.add)
                    arc = ep.tile([128, 4, S, 1], F32, tag="arc")
                    nc.vector.reciprocal(arc[:], asm[:])
                    nc.vector.tensor_mul(aex[:], aex[:], arc[:].to_broadcast(BSE))

                    # two-pass std over samples (ddof=1), mean over experts
                    sump = ep.tile([128, 4, E], F32, tag="sump")
                    st["sump"] = sump
                    nc.vector.tensor_reduce(
                        sump[:],
                        aex[:].rearrange("p j s e -> p j e s"),
                        axis=AxX,
                        op=Alu.add,
                    )

                def stage_d():
                    aex, sump = st["aex"], st["sump"]
                    for j in range(4):
                        nc.vector.scalar_tensor_tensor(
                            aex[:, j, :, :],
                            sump[:, j, None, :].to_broadcast([128, S, E]),
                            -1.0 / S,
                            aex[:, j, :, :],
                            op0=Alu.mult,
                            op1=Alu.add,
                        )
                    nc.vector.tensor_mul(aex[:], aex[:], aex[:])
                    vars_ = ep.tile([128, 4, E], F32, tag="vars")
                    nc.vector.tensor_reduce(
                        vars_[:],
                        aex[:].rearrange("p j s e -> p j e s"),
                        axis=AxX,
                        op=Alu.add,
                    )
                    stdt = ep.tile([128, 4, E], F32, tag="stdt")
                    nc.scalar.activation(stdt[:], vars_[:], Act.Sqrt, scale=1.0 / (S - 1))
                    usum = ep.tile([128, 4, 1], F32, tag="usum")
                    nc.vector.tensor_reduce(usum[:], stdt[:], axis=AxX, op=Alu.add)
                    nc.vector.tensor_scalar_mul(
                        up[:, t0 : t0 + 4],
                        usum[:].rearrange("p j one -> p (j one)"),
                        1.0 / E,
                    )

                def stage_e():
                    probs = st["probs"]
                    for j in range(4):
                        t = t0 + j
                        mv = eout.tile([128, 8], F32, tag="mv")
                        nc.vector.max(out=mv[:], in_=probs[:, j, :])
                        mi = eout.tile([128, 8], U32, tag="mi")
                        nc.vector.max_index(
                            out=mi[:], in_max=mv[:], in_values=probs[:, j, :]
                        )
                        mg = eout.tile([128, 1], F32, tag="mg")
                        nc.vector.tensor_scalar(
                            mg[:], up[:, t : t + 1], THR, None, op0=Alu.is_gt
                        )
                        mgi = eout.tile([128, 1], I32, tag="mgi")
                        nc.vector.tensor_scalar(
                            mgi[:], up[:, t : t + 1], THR, None, op0=Alu.is_gt
                        )
                        nc.vector.tensor_copy(pp[:, t, :], mv[:, :4])
                        nc.vector.tensor_scalar_mul(
                            pp[:, t, 2:4], pp[:, t, 2:4], mg[:]
                        )
                        mi32 = eout.tile([128, 4], I32, tag="mi32")
                        nc.vector.tensor_copy(mi32[:], mi[:, :4])
                        nc.vector.tensor_copy(ip[:, t, :2], mi32[:, :2])
                        nc.vector.memset(ip[:, t, 2:4], -1)
                        nc.vector.copy_predicated(
                            ip[:, t, 2:4], mgi[:].to_broadcast([128, 2]), mi32[:, 2:4]
                        )
                    nc.sync.dma_start(ip_d[:, t0 : t0 + 4, :], ip[:, t0 : t0 + 4, :])
                    nc.sync.dma_start(pp_d[:, t0 : t0 + 4, :], pp[:, t0 : t0 + 4, :])
                    nc.sync.dma_start(up_d[:, t0 : t0 + 4], up[:, t0 : t0 + 4])

                return [stage_a, stage_b, stage_c, stage_d, stage_e]

            pending = []  # deferred epilogue pieces from the previous block
            mm_q = []  # skewed L1 matmul emission

            def flush_mm():
                while mm_q:
                    mm_q.pop(0)()

            for blk in range(nblk):
                b0 = blk * 512
                h_ps = hps.tile([128, 512], F32, tag="hps")

                def make_mm(h_ps, xt_sb, k0):
                    def emit():
                        for kk in range(2):
                            k = k0 + kk
                            nc.tensor.matmul(
                                h_ps[:],
                                w1_sb[:, k, :],
                                xt_sb[:, kk, :],
                                start=(k == 0),
                                stop=(k == KCH - 1),
                            )

                    return emit

                for kc in range(4):
                    xn = []
                    for j in range(4):
                        xc = xpool.tile([128, 1024], F32, tag=f"x{j}")
                        nc.sync.dma_start(
                            xc[:],
                            x_d[
                                b0 + j * 128 : b0 + (j + 1) * 128,
                                kc * 1024 : (kc + 1) * 1024,
                            ],
                        )
                        xn.append(xc)
                    for dk2 in range(4):  # two k-chunks per 2-bank psum tile
                        xt_ps = tps.tile([128, 2, 512], F32, tag="tps")
                        for kk in range(2):
                            dk = dk2 * 2 + kk
                            for j in range(4):
                                nc.tensor.transpose(
                                    xt_ps[:, kk, j * 128 : (j + 1) * 128],
                                    xn[j][:, dk * 128 : (dk + 1) * 128],
                                    ident[:],
                                )
                        xt_sb = xtpool.tile([128, 2, 512], F32, tag="xt")
                        if dk2 % 2 == 0:
                            nc.vector.tensor_copy(xt_sb[:], xt_ps[:])
                        else:
                            nc.scalar.copy(xt_sb[:], xt_ps[:])
                        flush_mm()
                        mm_q.append(make_mm(h_ps, xt_sb, kc * 8 + dk2 * 2))
                        if pending:
                            pending.pop(0)()
                flush_mm()

                h_sb = hpool.tile([128, 512], F32, tag="hsb")
                nc.scalar.activation(
                    h_sb[:], h_ps[:], Act.Relu, bias=b1s[:], scale=SCALE
                )

                hm = []
                for pair in ((0, 1), (2, 3), (4,)):
                    mp_ps = tps.tile([128, 2, 512], F32, tag="tps")
                    for si, s in enumerate(pair):
                        m1n = m1pool.tile([128, 4, 128], F32, tag="m1n")
                        nc.scalar.dma_start(
                            m1n[:],
                            m1_d[s, b0 : b0 + 512, :].rearrange(
                                "(j p) h -> p j h", p=128
                            ),
                        )
                        for j in range(4):
                            nc.tensor.transpose(
                                mp_ps[:, si, j * 128 : (j + 1) * 128],
                                m1n[:, j, :],
                                ident[:],
                            )
                    m1t = m1tpool.tile([128, 2, 512], F32, tag="m1t")
                    np_ = len(pair)
                    nc.vector.tensor_scalar(
                        m1t[:, :np_, :], mp_ps[:, :np_, :], P_DROP, None, op0=Alu.is_ge
                    )
                    for si, s in enumerate(pair):
                        hms = hmpool.tile([128, 512], F32, tag=f"hm{s}")
                        nc.vector.tensor_mul(hms[:], h_sb[:], m1t[:, si, :])
                        hm.append(hms)

                lgb4 = ebig.tile([128, 4, S, E], F32, tag="lgb4")
                pending = [
                    make_lgb_part(blk, j, hm, lgb4) for j in range(4)
                ] + make_batch_stages(blk, lgb4)

            while pending:
                pending.pop(0)()

    nc.compile()
    if not nc.is_finalized():
        nc.finalize()
    return nc


_NC_CACHE = {}


def _get_nc(b_loc):
    if b_loc not in _NC_CACHE:
        _NC_CACHE[b_loc] = build(b_loc)
    return _NC_CACHE[b_loc]


def _unpack(res_c, b_loc):
    nt = b_loc // 128
    idx = np.ascontiguousarray(
        res_c["idx_pack"].transpose(1, 0, 2).reshape(b_loc, 4)
    ).astype(np.int32)
    prb = np.ascontiguousarray(
        res_c["prob_pack"].transpose(1, 0, 2).reshape(b_loc, 4)
    ).astype(np.float32)
    unc = np.ascontiguousarray(res_c["unc_pack"].transpose(1, 0).reshape(b_loc)).astype(
        np.float32
    )
    return idx, prb, unc


def run_sharded(x, W1, b1, W2, b2, mask1_u, mask2_u, n_cores=N_CORES, **kw):
    x = np.asarray(x, np.float32)
    W1 = np.asarray(W1, np.float32)
    b1 = np.asarray(b1, np.float32)
    W2 = np.asarray(W2, np.float32)
    b2 = np.asarray(b2, np.float32)
    mask1_u = np.asarray(mask1_u, np.float32)
    mask2_u = np.asarray(mask2_u, np.float32)

    B = x.shape[0]
    b_loc = B // n_cores
    nc = _get_nc(b_loc)
    in_maps = []
    for c in range(n_cores):
        sl = slice(c * b_loc, (c + 1) * b_loc)
        in_maps.append(
            {
                "xs": np.ascontiguousarray(x[sl]),
                "m1s": np.ascontiguousarray(mask1_u[:, sl, :]),
                "m2s": np.ascontiguousarray(mask2_u[:, sl, :]),
                "w1": W1,
                "b1": b1,
                "w2": W2,
                "b2": b2,
            }
        )
    res = run_bass_kernel_spmd(nc, in_maps, core_ids=list(range(n_cores)), **kw)
    outs = [_unpack(r, b_loc) for r in res.results]
    out_idx = np.concatenate([o[0] for o in outs], axis=0)
    out_probs = np.concatenate([o[1] for o in outs], axis=0)
    unc = np.concatenate([o[2] for o in outs], axis=0)
    return (out_idx, out_probs, unc), res


def kernel(x, W1, b1, W2, b2, mask1_u, mask2_u):
    (out_idx, out_probs, unc), _ = run_sharded(x, W1, b1, W2, b2, mask1_u, mask2_u)
    return out_idx, out_probs, unc


# revision 16
# speedup vs baseline: 1.0958x; 1.0958x over previous
"""Bayesian routing strategy (MoE routing) Bass kernel for 8 TRN2 NeuronCores.

Data-parallel over tokens: each core handles B/8 = 2048 tokens with
replicated router params (W1/b1/W2/b2). Math per core:
    h  = relu(x @ W1 + b1) * 1/(1-p)                    [hT layout: 128h x b]
    hm_s = h * (mask1_u[s] >= p)                        (5 samples)
    logits_s = (hm_s @ W2 + b2) * (mask2_u[s] >= p)/(1-p)   [128b x 5 x 64e]
    probs = softmax(mean_s logits); all_probs = softmax(logits_s)
    unc = mean_e std_s(all_probs, ddof=1)
    top4 of probs via DVE max8/max_index; extra 2 gated by unc > 0.3.

All matmuls fp32 (fp32r measured at ~1e-4 — too lossy for stable top-k).
x must be transposed (contraction dim on partitions): done on PE
(fp32 transpose = 2 cyc/row) interleaved with the fp32 matmuls.
"""

import sys

if "/opt/trn_rl_repo" not in sys.path:
    sys.path.insert(0, "/opt/trn_rl_repo")

import numpy as np

import concourse.bacc as bacc
import concourse.mybir as mybir
import concourse.tile as tile
from concourse.masks import make_identity
from concourse.bass_utils import run_bass_kernel_spmd

F32 = mybir.dt.float32
I32 = mybir.dt.int32
U32 = mybir.dt.uint32
Alu = mybir.AluOpType
Act = mybir.ActivationFunctionType
AxX = mybir.AxisListType.X

P_DROP = 0.3
SCALE = 1.0 / (1.0 - P_DROP)
THR = 0.3
S = 5
D = 4096
H = 128
E = 64
N_CORES = 8
B_FULL = 16384


def build(b_loc=B_FULL // N_CORES):
    """Build the per-core SPMD program. b_loc = tokens per core (mult of 512)."""
    nblk = b_loc // 512
    nt = b_loc // 128  # b-tiles per core
    KCH = D // 128  # 32 k-chunks
    nc = bacc.Bacc(None, target_bir_lowering=False, debug=False)

    x_d = nc.dram_tensor("xs", [b_loc, D], F32, kind="ExternalInput")
    m1_d = nc.dram_tensor("m1s", [S, b_loc, H], F32, kind="ExternalInput")
    m2_d = nc.dram_tensor("m2s", [S, b_loc, E], F32, kind="ExternalInput")
    w1_d = nc.dram_tensor("w1", [D, H], F32, kind="ExternalInput")
    b1_d = nc.dram_tensor("b1", [H], F32, kind="ExternalInput")
    w2_d = nc.dram_tensor("w2", [H, E], F32, kind="ExternalInput")
    b2_d = nc.dram_tensor("b2", [E], F32, kind="ExternalInput")

    ip_d = nc.dram_tensor("idx_pack", [128, nt, 4], I32, kind="ExternalOutput")
    pp_d = nc.dram_tensor("prob_pack", [128, nt, 4], F32, kind="ExternalOutput")
    up_d = nc.dram_tensor("unc_pack", [128, nt], F32, kind="ExternalOutput")

    with tile.TileContext(nc) as tc:
        with (
            tc.tile_pool(name="const", bufs=1) as cpool,
            tc.tile_pool(name="outp", bufs=1) as opool,
            tc.tile_pool(name="xn", bufs=3) as xpool,
            tc.tile_pool(name="xt", bufs=3) as xtpool,
            tc.tile_pool(name="hsb", bufs=2) as hpool,
            tc.tile_pool(name="m1n", bufs=2) as m1pool,
            tc.tile_pool(name="m1t", bufs=2) as m1tpool,
            tc.tile_pool(name="hm", bufs=2) as hmpool,
            tc.tile_pool(name="m2", bufs=2) as m2pool,
            tc.tile_pool(name="ep", bufs=2) as ep,
            tc.tile_pool(name="tps", bufs=2, space="PSUM") as tps,
            tc.tile_pool(name="hps", bufs=2, space="PSUM") as hps,
            tc.tile_pool(name="lgps", bufs=2, space="PSUM") as lgps,
        ):
            ident = cpool.tile([128, 128], F32, tag="ident")
            make_identity(nc, ident[:])

            w1_sb = cpool.tile([128, KCH, H], F32, tag="w1")
            nc.gpsimd.dma_start(w1_sb[:], w1_d[:].rearrange("(k p) h -> p k h", p=128))
            w2_sb = cpool.tile([128, E], F32, tag="w2")
            nc.scalar.dma_start(w2_sb[:], w2_d[:])
            b1_sb = cpool.tile([128, 1], F32, tag="b1")
            nc.scalar.dma_start(b1_sb[:], b1_d[:].rearrange("(p one) -> p one", one=1))
            b1s = cpool.tile([128, 1], F32, tag="b1s")
            nc.vector.tensor_scalar_mul(b1s[:], b1_sb[:], SCALE)
            b2_row = cpool.tile([1, E], F32, tag="b2row")
            nc.scalar.dma_start(b2_row[:], b2_d[:].rearrange("(one e) -> one e", one=1))
            b2_bc = cpool.tile([128, E], F32, tag="b2bc")
            nc.gpsimd.partition_broadcast(b2_bc[:], b2_row[:])
            b2s_bc = cpool.tile([128, E], F32, tag="b2sbc")
            nc.vector.tensor_scalar_mul(b2s_bc[:], b2_bc[:], SCALE)

            ip = opool.tile([128, nt, 4], I32, tag="ip")
            pp = opool.tile([128, nt, 4], F32, tag="pp")
            up = opool.tile([128, nt], F32, tag="up")

            copy_flip = [0]

            def psum_copy(dst, src):
                if copy_flip[0] % 2 == 0:
                    nc.vector.tensor_copy(dst, src)
                else:
                    nc.scalar.copy(dst, src)
                copy_flip[0] += 1

            def make_epilogue(t, hm):
                j = t % 4

                def emit():
                    lg_ps = lgps.tile([128, S, E], F32, tag="lgps")
                    for s in range(S):
                        nc.tensor.matmul(
                            lg_ps[:, s, :],
                            hm[s][:, j * 128 : (j + 1) * 128],
                            w2_sb[:],
                            start=True,
                            stop=True,
                        )
                    m2u = m2pool.tile([128, S, E], F32, tag="m2u")
                    nc.scalar.dma_start(
                        m2u[:],
                        m2_d[:, t * 128 : (t + 1) * 128, :].rearrange("s p e -> p s e"),
                    )
                    lgbs = ep.tile([128, S, E], F32, tag="lgbs")
                    nc.vector.scalar_tensor_tensor(
                        lgbs[:],
                        lg_ps[:],
                        SCALE,
                        b2s_bc[:][:, None, :].to_broadcast([128, S, E]),
                        op0=Alu.mult,
                        op1=Alu.add,
                    )
                    lgd = ep.tile([128, S, E], F32, tag="lgd")
                    nc.vector.scalar_tensor_tensor(
                        lgd[:], m2u[:], P_DROP, lgbs[:], op0=Alu.is_ge, op1=Alu.mult
                    )

                    # mean-softmax: probs = softmax(sum_s lgd / S)
                    ml = ep.tile([128, E], F32, tag="ml")
                    nc.vector.tensor_reduce(
                        ml[:], lgd[:].rearrange("p s e -> p e s"), axis=AxX, op=Alu.add
                    )
                    rmax = ep.tile([128, 1], F32, tag="rmax")
                    nc.vector.tensor_reduce(rmax[:], ml[:], axis=AxX, op=Alu.max)
                    ebias = ep.tile([128, 1], F32, tag="ebias")
                    nc.vector.tensor_scalar_mul(ebias[:], rmax[:], -1.0 / S)
                    pex = ep.tile([128, E], F32, tag="pex")
                    psm = ep.tile([128, 1], F32, tag="psm")
                    nc.scalar.activation(
                        pex[:], ml[:], Act.Exp, bias=ebias[:], scale=1.0 / S,
                        accum_out=psm[:],
                    )
                    prc = ep.tile([128, 1], F32, tag="prc")
                    nc.vector.reciprocal(prc[:], psm[:])
                    probs = ep.tile([128, E], F32, tag="probs")
                    nc.vector.tensor_scalar_mul(probs[:], pex[:], prc[:])

                    # per-sample softmax via per-s ACT exp with bias = -max_s
                    rmx = ep.tile([128, S], F32, tag="rmx")
                    nc.vector.tensor_reduce(rmx[:], lgd[:], axis=AxX, op=Alu.max)
                    nrmx = ep.tile([128, S], F32, tag="nrmx")
                    nc.vector.tensor_scalar_mul(nrmx[:], rmx[:], -1.0)
                    aex = ep.tile([128, S, E], F32, tag="aex")
                    asm = ep.tile([128, S], F32, tag="asm")
                    for s in range(S):
                        nc.scalar.activation(
                            aex[:, s, :], lgd[:, s, :], Act.Exp,
                            bias=nrmx[:, s : s + 1], accum_out=asm[:, s : s + 1],
                        )
                    arc = ep.tile([128, S, 1], F32, tag="arc")
                    nc.vector.reciprocal(arc[:], asm[:][:, :, None])
                    aprobs = ep.tile([128, S, E], F32, tag="aprobs")
                    nc.vector.tensor_mul(
                        aprobs[:], aex[:], arc[:].to_broadcast([128, S, E])
                    )

                    # two-pass std over samples (ddof=1), mean over experts
                    sump = ep.tile([128, E], F32, tag="sump")
                    nc.vector.tensor_reduce(
                        sump[:],
                        aprobs[:].rearrange("p s e -> p e s"),
                        axis=AxX,
                        op=Alu.add,
                    )
                    dev = ep.tile([128, S, E], F32, tag="dev")
                    nc.vector.scalar_tensor_tensor(
                        dev[:],
                        sump[:][:, None, :].to_broadcast([128, S, E]),
                        -1.0 / S,
                        aprobs[:],
                        op0=Alu.mult,
                        op1=Alu.add,
                    )
                    sq = ep.tile([128, S, E], F32, tag="sq")
                    nc.vector.tensor_mul(sq[:], dev[:], dev[:])
                    vars_ = ep.tile([128, E], F32, tag="vars")
                    nc.vector.tensor_reduce(
                        vars_[:], sq[:].rearrange("p s e -> p e s"), axis=AxX, op=Alu.add
                    )
                    stdt = ep.tile([128, E], F32, tag="stdt")
                    usum = ep.tile([128, 1], F32, tag="usum")
                    nc.scalar.activation(
                        stdt[:], vars_[:], Act.Sqrt, scale=1.0 / (S - 1),
                        accum_out=usum[:],
                    )
                    nc.vector.tensor_scalar_mul(up[:, t : t + 1], usum[:], 1.0 / E)

                    # top-k
                    mv = ep.tile([128, 8], F32, tag="mv")
                    nc.vector.max(out=mv[:], in_=probs[:])
                    mi = ep.tile([128, 8], U32, tag="mi")
                    nc.vector.max_index(out=mi[:], in_max=mv[:], in_values=probs[:])

                    mg = ep.tile([128, 1], F32, tag="mg")
                    nc.vector.tensor_scalar(
                        mg[:], up[:, t : t + 1], THR, None, op0=Alu.is_gt
                    )
                    mgi = ep.tile([128, 1], I32, tag="mgi")
                    nc.vector.tensor_scalar(
                        mgi[:], up[:, t : t + 1], THR, None, op0=Alu.is_gt
                    )
                    nc.vector.tensor_copy(pp[:, t, :], mv[:, :4])
                    nc.vector.tensor_scalar_mul(pp[:, t, 2:4], pp[:, t, 2:4], mg[:])
                    mi32 = ep.tile([128, 4], I32, tag="mi32")
                    nc.vector.tensor_copy(mi32[:], mi[:, :4])
                    nc.vector.tensor_copy(ip[:, t, :2], mi32[:, :2])
                    nc.vector.memset(ip[:, t, 2:4], -1)
                    nc.vector.copy_predicated(
                        ip[:, t, 2:4], mgi[:].to_broadcast([128, 2]), mi32[:, 2:4]
                    )
                    if j == 3:
                        t0 = t - 3
                        nc.sync.dma_start(
                            ip_d[:, t0 : t0 + 4, :], ip[:, t0 : t0 + 4, :]
                        )
                        nc.sync.dma_start(
                            pp_d[:, t0 : t0 + 4, :], pp[:, t0 : t0 + 4, :]
                        )
                        nc.sync.dma_start(up_d[:, t0 : t0 + 4], up[:, t0 : t0 + 4])

                return emit

            pending = []  # deferred per-tile epilogues from the previous block
            mm_q = []  # skewed L1 matmul emission

            def flush_mm():
                while mm_q:
                    mm_q.pop(0)()

            for blk in range(nblk):
                b0 = blk * 512
                h_ps = hps.tile([128, 512], F32, tag="hps")

                def make_mm(h_ps, xt_sb, k0):
                    def emit():
                        for kk in range(2):
                            k = k0 + kk
                            nc.tensor.matmul(
                                h_ps[:],
                                w1_sb[:, k, :],
                                xt_sb[:, kk, :],
                                start=(k == 0),
                                stop=(k == KCH - 1),
                            )

                    return emit

                for kc in range(4):
                    xn = []
                    for j in range(4):
                        xc = xpool.tile([128, 1024], F32, tag=f"x{j}")
                        nc.sync.dma_start(
                            xc[:],
                            x_d[
                                b0 + j * 128 : b0 + (j + 1) * 128,
                                kc * 1024 : (kc + 1) * 1024,
                            ],
                        )
                        xn.append(xc)
                    for dk2 in range(4):  # two k-chunks per 2-bank psum tile
                        xt_ps = tps.tile([128, 2, 512], F32, tag="tps")
                        for kk in range(2):
                            dk = dk2 * 2 + kk
                            for j in range(4):
                                nc.tensor.transpose(
                                    xt_ps[:, kk, j * 128 : (j + 1) * 128],
                                    xn[j][:, dk * 128 : (dk + 1) * 128],
                                    ident[:],
                                )
                        xt_sb = xtpool.tile([128, 2, 512], F32, tag="xt")
                        psum_copy(xt_sb[:], xt_ps[:])
                        flush_mm()
                        mm_q.append(make_mm(h_ps, xt_sb, kc * 8 + dk2 * 2))
                    if pending:
                        pending.pop(0)()
                flush_mm()
                while pending:
                    pending.pop(0)()

                h_sb = hpool.tile([128, 512], F32, tag="hsb")
                nc.scalar.activation(
                    h_sb[:], h_ps[:], Act.Relu, bias=b1s[:], scale=SCALE
                )

                hm = []
                for pair in ((0, 1), (2, 3), (4,)):
                    mp_ps = tps.tile([128, 2, 512], F32, tag="tps")
                    for si, s in enumerate(pair):
                        m1n = m1pool.tile([128, 4, 128], F32, tag="m1n")
                        nc.scalar.dma_start(
                            m1n[:],
                            m1_d[s, b0 : b0 + 512, :].rearrange(
                                "(j p) h -> p j h", p=128
                            ),
                        )
                        for j in range(4):
                            nc.tensor.transpose(
                                mp_ps[:, si, j * 128 : (j + 1) * 128],
                                m1n[:, j, :],
                                ident[:],
                            )
                    m1t = m1tpool.tile([128, 2, 512], F32, tag="m1t")
                    np_ = len(pair)
                    nc.vector.tensor_scalar(
                        m1t[:, :np_, :], mp_ps[:, :np_, :], P_DROP, None, op0=Alu.is_ge
                    )
                    for si, s in enumerate(pair):
                        hms = hmpool.tile([128, 512], F32, tag=f"hm{s}")
                        nc.vector.tensor_mul(hms[:], h_sb[:], m1t[:, si, :])
                        hm.append(hms)

                pending = [make_epilogue(blk * 4 + j, hm) for j in range(4)]

            while pending:
                pending.pop(0)()

    nc.compile()
    if not nc.is_finalized():
        nc.finalize()
    return nc


_NC_CACHE = {}


def _get_nc(b_loc):
    if b_loc not in _NC_CACHE:
        _NC_CACHE[b_loc] = build(b_loc)
    return _NC_CACHE[b_loc]


def _unpack(res_c, b_loc):
    nt = b_loc // 128
    idx = np.ascontiguousarray(
        res_c["idx_pack"].transpose(1, 0, 2).reshape(b_loc, 4)
    ).astype(np.int32)
    prb = np.ascontiguousarray(
        res_c["prob_pack"].transpose(1, 0, 2).reshape(b_loc, 4)
    ).astype(np.float32)
    unc = np.ascontiguousarray(res_c["unc_pack"].transpose(1, 0).reshape(b_loc)).astype(
        np.float32
    )
    return idx, prb, unc


def run_sharded(x, W1, b1, W2, b2, mask1_u, mask2_u, n_cores=N_CORES, **kw):
    x = np.asarray(x, np.float32)
    W1 = np.asarray(W1, np.float32)
    b1 = np.asarray(b1, np.float32)
    W2 = np.asarray(W2, np.float32)
    b2 = np.asarray(b2, np.float32)
    mask1_u = np.asarray(mask1_u, np.float32)
    mask2_u = np.asarray(mask2_u, np.float32)

    B = x.shape[0]
    b_loc = B // n_cores
    nc = _get_nc(b_loc)
    in_maps = []
    for c in range(n_cores):
        sl = slice(c * b_loc, (c + 1) * b_loc)
        in_maps.append(
            {
                "xs": np.ascontiguousarray(x[sl]),
                "m1s": np.ascontiguousarray(mask1_u[:, sl, :]),
                "m2s": np.ascontiguousarray(mask2_u[:, sl, :]),
                "w1": W1,
                "b1": b1,
                "w2": W2,
                "b2": b2,
            }
        )
    res = run_bass_kernel_spmd(nc, in_maps, core_ids=list(range(n_cores)), **kw)
    outs = [_unpack(r, b_loc) for r in res.results]
    out_idx = np.concatenate([o[0] for o in outs], axis=0)
    out_probs = np.concatenate([o[1] for o in outs], axis=0)
    unc = np.concatenate([o[2] for o in outs], axis=0)
    return (out_idx, out_probs, unc), res


def kernel(x, W1, b1, W2, b2, mask1_u, mask2_u):
    (out_idx, out_probs, unc), _ = run_sharded(x, W1, b1, W2, b2, mask1_u, mask2_u)
    return out_idx, out_probs, unc


# revision 18
# speedup vs baseline: 1.1696x; 1.0674x over previous
"""Bayesian routing strategy (MoE routing) Bass kernel for 8 TRN2 NeuronCores.

Data-parallel over tokens: each core handles B/8 = 2048 tokens with
replicated router params (W1/b1/W2/b2). Math per core:
    h  = relu(x @ W1 + b1) * 1/(1-p)                    [hT layout: 128h x b]
    hm_s = h * (mask1_u[s] >= p)                        (5 samples)
    logits_s = (hm_s @ W2 + b2) * (mask2_u[s] >= p)/(1-p)   [128b x 5 x 64e]
    probs = softmax(mean_s logits); all_probs = softmax(logits_s)
    unc = mean_e std_s(all_probs, ddof=1)
    top4 of probs via DVE max8/max_index; extra 2 gated by unc > 0.3.

All matmuls fp32 (fp32r measured at ~1e-4 — too lossy for stable top-k).
x must be transposed (contraction dim on partitions): done on PE
(fp32 transpose = 2 cyc/row) interleaved with the fp32 matmuls.
"""

import sys

if "/opt/trn_rl_repo" not in sys.path:
    sys.path.insert(0, "/opt/trn_rl_repo")

import numpy as np

import concourse.bacc as bacc
import concourse.mybir as mybir
import concourse.tile as tile
from concourse.masks import make_identity
from concourse.bass_utils import run_bass_kernel_spmd

F32 = mybir.dt.float32
I32 = mybir.dt.int32
U32 = mybir.dt.uint32
Alu = mybir.AluOpType
Act = mybir.ActivationFunctionType
AxX = mybir.AxisListType.X

P_DROP = 0.3
SCALE = 1.0 / (1.0 - P_DROP)
THR = 0.3
S = 5
D = 4096
H = 128
E = 64
N_CORES = 8
B_FULL = 16384


def build(b_loc=B_FULL // N_CORES):
    """Build the per-core SPMD program. b_loc = tokens per core (mult of 512)."""
    nblk = b_loc // 512
    nt = b_loc // 128  # b-tiles per core
    KCH = D // 128  # 32 k-chunks
    nc = bacc.Bacc(None, target_bir_lowering=False, debug=False)

    x_d = nc.dram_tensor("xs", [b_loc, D], F32, kind="ExternalInput")
    m1_d = nc.dram_tensor("m1s", [S, b_loc, H], F32, kind="ExternalInput")
    m2_d = nc.dram_tensor("m2s", [S, b_loc, E], F32, kind="ExternalInput")
    w1_d = nc.dram_tensor("w1", [D, H], F32, kind="ExternalInput")
    b1_d = nc.dram_tensor("b1", [H], F32, kind="ExternalInput")
    w2_d = nc.dram_tensor("w2", [H, E], F32, kind="ExternalInput")
    b2_d = nc.dram_tensor("b2", [E], F32, kind="ExternalInput")

    ip_d = nc.dram_tensor("idx_pack", [128, nt, 4], I32, kind="ExternalOutput")
    pp_d = nc.dram_tensor("prob_pack", [128, nt, 4], F32, kind="ExternalOutput")
    up_d = nc.dram_tensor("unc_pack", [128, nt], F32, kind="ExternalOutput")

    with tile.TileContext(nc) as tc:
        with (
            tc.tile_pool(name="const", bufs=1) as cpool,
            tc.tile_pool(name="outp", bufs=1) as opool,
            tc.tile_pool(name="xn", bufs=3) as xpool,
            tc.tile_pool(name="xt", bufs=3) as xtpool,
            tc.tile_pool(name="hsb", bufs=2) as hpool,
            tc.tile_pool(name="m1n", bufs=2) as m1pool,
            tc.tile_pool(name="m1t", bufs=2) as m1tpool,
            tc.tile_pool(name="hm", bufs=2) as hmpool,
            tc.tile_pool(name="m2", bufs=2) as m2pool,
            tc.tile_pool(name="ep", bufs=2) as ep,
            tc.tile_pool(name="tps", bufs=2, space="PSUM") as tps,
            tc.tile_pool(name="m1ps", bufs=1, space="PSUM") as m1ps,
            tc.tile_pool(name="hps", bufs=1, space="PSUM") as hps,
            tc.tile_pool(name="lgps", bufs=1, space="PSUM") as lgps,
        ):
            ident = cpool.tile([128, 128], F32, tag="ident")
            make_identity(nc, ident[:])

            w1_sb = cpool.tile([128, KCH, H], F32, tag="w1")
            nc.gpsimd.dma_start(w1_sb[:], w1_d[:].rearrange("(k p) h -> p k h", p=128))
            w2_sb = cpool.tile([128, E], F32, tag="w2")
            nc.scalar.dma_start(w2_sb[:], w2_d[:])
            b1_sb = cpool.tile([128, 1], F32, tag="b1")
            nc.scalar.dma_start(b1_sb[:], b1_d[:].rearrange("(p one) -> p one", one=1))
            b1s = cpool.tile([128, 1], F32, tag="b1s")
            nc.vector.tensor_scalar_mul(b1s[:], b1_sb[:], SCALE)
            b2_row = cpool.tile([1, E], F32, tag="b2row")
            nc.scalar.dma_start(b2_row[:], b2_d[:].rearrange("(one e) -> one e", one=1))
            b2_bc = cpool.tile([128, E], F32, tag="b2bc")
            nc.gpsimd.partition_broadcast(b2_bc[:], b2_row[:])
            b2s_bc = cpool.tile([128, E], F32, tag="b2sbc")
            nc.vector.tensor_scalar_mul(b2s_bc[:], b2_bc[:], SCALE)

            ip = opool.tile([128, nt, 4], I32, tag="ip")
            pp = opool.tile([128, nt, 4], F32, tag="pp")
            up = opool.tile([128, nt], F32, tag="up")

            copy_flip = [0]

            def psum_copy(dst, src):
                if copy_flip[0] % 2 == 0:
                    nc.vector.tensor_copy(dst, src)
                else:
                    nc.scalar.copy(dst, src)
                copy_flip[0] += 1

            def make_epilogue(t, hm):
                j = t % 4

                def emit():
                    lg_ps = lgps.tile([128, S, E], F32, tag="lgps")
                    for s in range(S):
                        nc.tensor.matmul(
                            lg_ps[:, s, :],
                            hm[s][:, j * 128 : (j + 1) * 128],
                            w2_sb[:],
                            start=True,
                            stop=True,
                        )
                    m2u = m2pool.tile([128, S, E], F32, tag="m2u")
                    nc.scalar.dma_start(
                        m2u[:],
                        m2_d[:, t * 128 : (t + 1) * 128, :].rearrange("s p e -> p s e"),
                    )
                    lgbs = ep.tile([128, S, E], F32, tag="lgbs")
                    nc.vector.scalar_tensor_tensor(
                        lgbs[:],
                        lg_ps[:],
                        SCALE,
                        b2s_bc[:][:, None, :].to_broadcast([128, S, E]),
                        op0=Alu.mult,
                        op1=Alu.add,
                    )
                    lgd = ep.tile([128, S, E], F32, tag="lgd")
                    nc.vector.scalar_tensor_tensor(
                        lgd[:], m2u[:], P_DROP, lgbs[:], op0=Alu.is_ge, op1=Alu.mult
                    )

                    # mean-softmax: probs = softmax(sum_s lgd / S)
                    ml = ep.tile([128, E], F32, tag="ml")
                    nc.vector.tensor_reduce(
                        ml[:], lgd[:].rearrange("p s e -> p e s"), axis=AxX, op=Alu.add
                    )
                    rmax = ep.tile([128, 1], F32, tag="rmax")
                    nc.vector.tensor_reduce(rmax[:], ml[:], axis=AxX, op=Alu.max)
                    ebias = ep.tile([128, 1], F32, tag="ebias")
                    nc.vector.tensor_scalar_mul(ebias[:], rmax[:], -1.0 / S)
                    pex = ep.tile([128, E], F32, tag="pex")
                    psm = ep.tile([128, 1], F32, tag="psm")
                    nc.scalar.activation(
                        pex[:], ml[:], Act.Exp, bias=ebias[:], scale=1.0 / S,
                        accum_out=psm[:],
                    )
                    prc = ep.tile([128, 1], F32, tag="prc")
                    nc.vector.reciprocal(prc[:], psm[:])
                    probs = ep.tile([128, E], F32, tag="probs")
                    nc.vector.tensor_scalar_mul(probs[:], pex[:], prc[:])

                    # per-sample softmax via per-s ACT exp with bias = -max_s
                    rmx = ep.tile([128, S], F32, tag="rmx")
                    nc.vector.tensor_reduce(rmx[:], lgd[:], axis=AxX, op=Alu.max)
                    nrmx = ep.tile([128, S], F32, tag="nrmx")
                    nc.vector.tensor_scalar_mul(nrmx[:], rmx[:], -1.0)
                    aex = ep.tile([128, S, E], F32, tag="aex")
                    asm = ep.tile([128, S], F32, tag="asm")
                    for s in range(S):
                        nc.scalar.activation(
                            aex[:, s, :], lgd[:, s, :], Act.Exp,
                            bias=nrmx[:, s : s + 1], accum_out=asm[:, s : s + 1],
                        )
                    arc = ep.tile([128, S, 1], F32, tag="arc")
                    nc.vector.reciprocal(arc[:], asm[:][:, :, None])
                    aprobs = ep.tile([128, S, E], F32, tag="aprobs")
                    nc.vector.tensor_mul(
                        aprobs[:], aex[:], arc[:].to_broadcast([128, S, E])
                    )

                    # two-pass std over samples (ddof=1), mean over experts
                    sump = ep.tile([128, E], F32, tag="sump")
                    nc.vector.tensor_reduce(
                        sump[:],
                        aprobs[:].rearrange("p s e -> p e s"),
                        axis=AxX,
                        op=Alu.add,
                    )
                    dev = ep.tile([128, S, E], F32, tag="dev")
                    nc.vector.scalar_tensor_tensor(
                        dev[:],
                        sump[:][:, None, :].to_broadcast([128, S, E]),
                        -1.0 / S,
                        aprobs[:],
                        op0=Alu.mult,
                        op1=Alu.add,
                    )
                    sq = ep.tile([128, S, E], F32, tag="sq")
                    nc.vector.tensor_mul(sq[:], dev[:], dev[:])
                    vars_ = ep.tile([128, E], F32, tag="vars")
                    nc.vector.tensor_reduce(
                        vars_[:], sq[:].rearrange("p s e -> p e s"), axis=AxX, op=Alu.add
                    )
                    stdt = ep.tile([128, E], F32, tag="stdt")
                    usum = ep.tile([128, 1], F32, tag="usum")
                    nc.scalar.activation(
                        stdt[:], vars_[:], Act.Sqrt, scale=1.0 / (S - 1),
                        accum_out=usum[:],
                    )
                    nc.vector.tensor_scalar_mul(up[:, t : t + 1], usum[:], 1.0 / E)

                    # top-k
                    mv = ep.tile([128, 8], F32, tag="mv")
                    nc.vector.max(out=mv[:], in_=probs[:])
                    mi = ep.tile([128, 8], U32, tag="mi")
                    nc.vector.max_index(out=mi[:], in_max=mv[:], in_values=probs[:])

                    mg = ep.tile([128, 1], F32, tag="mg")
                    nc.vector.tensor_scalar(
                        mg[:], up[:, t : t + 1], THR, None, op0=Alu.is_gt
                    )
                    mgi = ep.tile([128, 1], I32, tag="mgi")
                    nc.vector.tensor_scalar(
                        mgi[:], up[:, t : t + 1], THR, None, op0=Alu.is_gt
                    )
                    nc.vector.tensor_copy(pp[:, t, :], mv[:, :4])
                    nc.vector.tensor_scalar_mul(pp[:, t, 2:4], pp[:, t, 2:4], mg[:])
                    mi32 = ep.tile([128, 4], I32, tag="mi32")
                    nc.vector.tensor_copy(mi32[:], mi[:, :4])
                    nc.vector.tensor_copy(ip[:, t, :2], mi32[:, :2])
                    nc.vector.memset(ip[:, t, 2:4], -1)
                    nc.vector.copy_predicated(
                        ip[:, t, 2:4], mgi[:].to_broadcast([128, 2]), mi32[:, 2:4]
                    )
                    if j == 3:
                        t0 = t - 3
                        nc.sync.dma_start(
                            ip_d[:, t0 : t0 + 4, :], ip[:, t0 : t0 + 4, :]
                        )
                        nc.sync.dma_start(
                            pp_d[:, t0 : t0 + 4, :], pp[:, t0 : t0 + 4, :]
                        )
                        nc.sync.dma_start(up_d[:, t0 : t0 + 4], up[:, t0 : t0 + 4])

                return emit

            pending = []  # deferred per-tile epilogues from the previous block
            mm_q = []  # skewed L1 matmul emission

            def flush_mm():
                while mm_q:
                    mm_q.pop(0)()

            for blk in range(nblk):
                b0 = blk * 512
                h_ps = hps.tile([128, 512], F32, tag="hps")

                def make_mm(h_ps, xt_sb, k0):
                    def emit():
                        for kk in range(2):
                            k = k0 + kk
                            nc.tensor.matmul(
                                h_ps[:],
                                w1_sb[:, k, :],
                                xt_sb[:, kk, :],
                                start=(k == 0),
                                stop=(k == KCH - 1),
                            )

                    return emit

                def make_m1_piece(pair, b0):
                    def emit():
                        mp_ps = m1ps.tile([128, 2, 512], F32, tag="m1ps")
                        for si, s in enumerate(pair):
                            m1n = m1pool.tile([128, 4, 128], F32, tag="m1n")
                            nc.scalar.dma_start(
                                m1n[:],
                                m1_d[s, b0 : b0 + 512, :].rearrange(
                                    "(j p) h -> p j h", p=128
                                ),
                            )
                            for j in range(4):
                                nc.tensor.transpose(
                                    mp_ps[:, si, j * 128 : (j + 1) * 128],
                                    m1n[:, j, :],
                                    ident[:],
                                )
                        m1t = m1tpool.tile([128, 2, 512], F32, tag="m1t")
                        np_ = len(pair)
                        nc.vector.tensor_scalar(
                            m1t[:, :np_, :],
                            mp_ps[:, :np_, :],
                            P_DROP,
                            None,
                            op0=Alu.is_ge,
                        )
                        m1t_parts.append((m1t, np_))

                    return emit

                m1t_parts = []
                m1_q = [make_m1_piece(p, b0) for p in ((0, 1), (2, 3), (4,))]

                for kc in range(4):
                    xn = []
                    for j in range(4):
                        xc = xpool.tile([128, 1024], F32, tag=f"x{j}")
                        nc.sync.dma_start(
                            xc[:],
                            x_d[
                                b0 + j * 128 : b0 + (j + 1) * 128,
                                kc * 1024 : (kc + 1) * 1024,
                            ],
                        )
                        xn.append(xc)
                    for dk2 in range(4):  # two k-chunks per 2-bank psum tile
                        xt_ps = tps.tile([128, 2, 512], F32, tag="tps")
                        for kk in range(2):
                            dk = dk2 * 2 + kk
                            for j in range(4):
                                nc.tensor.transpose(
                                    xt_ps[:, kk, j * 128 : (j + 1) * 128],
                                    xn[j][:, dk * 128 : (dk + 1) * 128],
                                    ident[:],
                                )
                        xt_sb = xtpool.tile([128, 2, 512], F32, tag="xt")
                        psum_copy(xt_sb[:], xt_ps[:])
                        flush_mm()
                        mm_q.append(make_mm(h_ps, xt_sb, kc * 8 + dk2 * 2))
                        if dk2 == 1 and m1_q:
                            m1_q.pop(0)()
                        elif dk2 == 3 and pending:
                            pending.pop(0)()
                flush_mm()
                while m1_q:
                    m1_q.pop(0)()
                while pending:
                    pending.pop(0)()

                h_sb = hpool.tile([128, 512], F32, tag="hsb")
                nc.scalar.activation(
                    h_sb[:], h_ps[:], Act.Relu, bias=b1s[:], scale=SCALE
                )

                hm = []
                for si_s in m1t_parts:
                    m1t, np_ = si_s
                    for si in range(np_):
                        s = len(hm)
                        hms = hmpool.tile([128, 512], F32, tag=f"hm{s}")
                        nc.vector.tensor_mul(hms[:], h_sb[:], m1t[:, si, :])
                        hm.append(hms)

                pending = [make_epilogue(blk * 4 + j, hm) for j in range(4)]

            while pending:
                pending.pop(0)()

    nc.compile()
    if not nc.is_finalized():
        nc.finalize()
    return nc


_NC_CACHE = {}


def _get_nc(b_loc):
    if b_loc not in _NC_CACHE:
        _NC_CACHE[b_loc] = build(b_loc)
    return _NC_CACHE[b_loc]


def _unpack(res_c, b_loc):
    nt = b_loc // 128
    idx = np.ascontiguousarray(
        res_c["idx_pack"].transpose(1, 0, 2).reshape(b_loc, 4)
    ).astype(np.int32)
    prb = np.ascontiguousarray(
        res_c["prob_pack"].transpose(1, 0, 2).reshape(b_loc, 4)
    ).astype(np.float32)
    unc = np.ascontiguousarray(res_c["unc_pack"].transpose(1, 0).reshape(b_loc)).astype(
        np.float32
    )
    return idx, prb, unc


def run_sharded(x, W1, b1, W2, b2, mask1_u, mask2_u, n_cores=N_CORES, **kw):
    x = np.asarray(x, np.float32)
    W1 = np.asarray(W1, np.float32)
    b1 = np.asarray(b1, np.float32)
    W2 = np.asarray(W2, np.float32)
    b2 = np.asarray(b2, np.float32)
    mask1_u = np.asarray(mask1_u, np.float32)
    mask2_u = np.asarray(mask2_u, np.float32)

    B = x.shape[0]
    b_loc = B // n_cores
    nc = _get_nc(b_loc)
    in_maps = []
    for c in range(n_cores):
        sl = slice(c * b_loc, (c + 1) * b_loc)
        in_maps.append(
            {
                "xs": np.ascontiguousarray(x[sl]),
                "m1s": np.ascontiguousarray(mask1_u[:, sl, :]),
                "m2s": np.ascontiguousarray(mask2_u[:, sl, :]),
                "w1": W1,
                "b1": b1,
                "w2": W2,
                "b2": b2,
            }
        )
    res = run_bass_kernel_spmd(nc, in_maps, core_ids=list(range(n_cores)), **kw)
    outs = [_unpack(r, b_loc) for r in res.results]
    out_idx = np.concatenate([o[0] for o in outs], axis=0)
    out_probs = np.concatenate([o[1] for o in outs], axis=0)
    unc = np.concatenate([o[2] for o in outs], axis=0)
    return (out_idx, out_probs, unc), res


def kernel(x, W1, b1, W2, b2, mask1_u, mask2_u):
    (out_idx, out_probs, unc), _ = run_sharded(x, W1, b1, W2, b2, mask1_u, mask2_u)
    return out_idx, out_probs, unc


# revision 19
# speedup vs baseline: 1.1809x; 1.0096x over previous
"""Bayesian routing strategy (MoE routing) Bass kernel for 8 TRN2 NeuronCores.

Data-parallel over tokens: each core handles B/8 = 2048 tokens with
replicated router params (W1/b1/W2/b2). Math per core:
    h  = relu(x @ W1 + b1) * 1/(1-p)                    [hT layout: 128h x b]
    hm_s = h * (mask1_u[s] >= p)                        (5 samples)
    logits_s = (hm_s @ W2 + b2) * (mask2_u[s] >= p)/(1-p)   [128b x 5 x 64e]
    probs = softmax(mean_s logits); all_probs = softmax(logits_s)
    unc = mean_e std_s(all_probs, ddof=1)
    top4 of probs via DVE max8/max_index; extra 2 gated by unc > 0.3.

All matmuls fp32 (fp32r measured at ~1e-4 — too lossy for stable top-k).
x must be transposed (contraction dim on partitions): done on PE
(fp32 transpose = 2 cyc/row) interleaved with the fp32 matmuls.
"""

import sys

if "/opt/trn_rl_repo" not in sys.path:
    sys.path.insert(0, "/opt/trn_rl_repo")

import numpy as np

import concourse.bacc as bacc
import concourse.mybir as mybir
import concourse.tile as tile
from concourse.masks import make_identity
from concourse.bass_utils import run_bass_kernel_spmd

F32 = mybir.dt.float32
I32 = mybir.dt.int32
U32 = mybir.dt.uint32
Alu = mybir.AluOpType
Act = mybir.ActivationFunctionType
AxX = mybir.AxisListType.X

P_DROP = 0.3
SCALE = 1.0 / (1.0 - P_DROP)
THR = 0.3
S = 5
D = 4096
H = 128
E = 64
N_CORES = 8
B_FULL = 16384


def build(b_loc=B_FULL // N_CORES):
    """Build the per-core SPMD program. b_loc = tokens per core (mult of 512)."""
    nblk = b_loc // 512
    nt = b_loc // 128  # b-tiles per core
    KCH = D // 128  # 32 k-chunks
    nc = bacc.Bacc(None, target_bir_lowering=False, debug=False)

    x_d = nc.dram_tensor("xs", [b_loc, D], F32, kind="ExternalInput")
    m1_d = nc.dram_tensor("m1s", [S, b_loc, H], F32, kind="ExternalInput")
    m2_d = nc.dram_tensor("m2s", [S, b_loc, E], F32, kind="ExternalInput")
    w1_d = nc.dram_tensor("w1", [D, H], F32, kind="ExternalInput")
    b1_d = nc.dram_tensor("b1", [H], F32, kind="ExternalInput")
    w2_d = nc.dram_tensor("w2", [H, E], F32, kind="ExternalInput")
    b2_d = nc.dram_tensor("b2", [E], F32, kind="ExternalInput")

    ip_d = nc.dram_tensor("idx_pack", [128, nt, 4], I32, kind="ExternalOutput")
    pp_d = nc.dram_tensor("prob_pack", [128, nt, 4], F32, kind="ExternalOutput")
    up_d = nc.dram_tensor("unc_pack", [128, nt], F32, kind="ExternalOutput")

    with tile.TileContext(nc) as tc:
        with (
            tc.tile_pool(name="const", bufs=1) as cpool,
            tc.tile_pool(name="outp", bufs=1) as opool,
            tc.tile_pool(name="xn", bufs=6) as xpool,
            tc.tile_pool(name="xt", bufs=3) as xtpool,
            tc.tile_pool(name="hsb", bufs=2) as hpool,
            tc.tile_pool(name="m1n", bufs=2) as m1pool,
            tc.tile_pool(name="m1t", bufs=2) as m1tpool,
            tc.tile_pool(name="hm", bufs=2) as hmpool,
            tc.tile_pool(name="m2", bufs=2) as m2pool,
            tc.tile_pool(name="ep", bufs=2) as ep,
            tc.tile_pool(name="tps", bufs=2, space="PSUM") as tps,
            tc.tile_pool(name="m1ps", bufs=1, space="PSUM") as m1ps,
            tc.tile_pool(name="hps", bufs=1, space="PSUM") as hps,
            tc.tile_pool(name="lgps", bufs=1, space="PSUM") as lgps,
        ):
            ident = cpool.tile([128, 128], F32, tag="ident")
            make_identity(nc, ident[:])

            w1_sb = cpool.tile([128, KCH, H], F32, tag="w1")
            nc.gpsimd.dma_start(w1_sb[:], w1_d[:].rearrange("(k p) h -> p k h", p=128))
            w2_sb = cpool.tile([128, E], F32, tag="w2")
            nc.scalar.dma_start(w2_sb[:], w2_d[:])
            b1s = cpool.tile([128, 1], F32, tag="b1s")
            b2s_bc = cpool.tile([128, E], F32, tag="b2sbc")

            def param_prep():
                b1_sb = cpool.tile([128, 1], F32, tag="b1")
                nc.scalar.dma_start(
                    b1_sb[:], b1_d[:].rearrange("(p one) -> p one", one=1)
                )
                nc.vector.tensor_scalar_mul(b1s[:], b1_sb[:], SCALE)
                b2_row = cpool.tile([1, E], F32, tag="b2row")
                nc.scalar.dma_start(
                    b2_row[:], b2_d[:].rearrange("(one e) -> one e", one=1)
                )
                b2_bc = cpool.tile([128, E], F32, tag="b2bc")
                nc.gpsimd.partition_broadcast(b2_bc[:], b2_row[:])
                nc.vector.tensor_scalar_mul(b2s_bc[:], b2_bc[:], SCALE)

            ip = opool.tile([128, nt, 4], I32, tag="ip")
            pp = opool.tile([128, nt, 4], F32, tag="pp")
            up = opool.tile([128, nt], F32, tag="up")

            copy_flip = [0]

            def psum_copy(dst, src):
                if copy_flip[0] % 2 == 0:
                    nc.vector.tensor_copy(dst, src)
                else:
                    nc.scalar.copy(dst, src)
                copy_flip[0] += 1

            def make_epilogue(t, hm):
                j = t % 4

                def emit():
                    lg_ps = lgps.tile([128, S, E], F32, tag="lgps")
                    for s in range(S):
                        nc.tensor.matmul(
                            lg_ps[:, s, :],
                            hm[s][:, j * 128 : (j + 1) * 128],
                            w2_sb[:],
                            start=True,
                            stop=True,
                        )
                    m2u = m2pool.tile([128, S, E], F32, tag="m2u")
                    nc.scalar.dma_start(
                        m2u[:],
                        m2_d[:, t * 128 : (t + 1) * 128, :].rearrange("s p e -> p s e"),
                    )
                    lgbs = ep.tile([128, S, E], F32, tag="lgbs")
                    nc.vector.scalar_tensor_tensor(
                        lgbs[:],
                        lg_ps[:],
                        SCALE,
                        b2s_bc[:][:, None, :].to_broadcast([128, S, E]),
                        op0=Alu.mult,
                        op1=Alu.add,
                    )
                    lgd = ep.tile([128, S, E], F32, tag="lgd")
                    nc.vector.scalar_tensor_tensor(
                        lgd[:], m2u[:], P_DROP, lgbs[:], op0=Alu.is_ge, op1=Alu.mult
                    )

                    # mean-softmax: probs = softmax(sum_s lgd / S)
                    ml = ep.tile([128, E], F32, tag="ml")
                    nc.vector.tensor_reduce(
                        ml[:], lgd[:].rearrange("p s e -> p e s"), axis=AxX, op=Alu.add
                    )
                    rmax = ep.tile([128, 1], F32, tag="rmax")
                    nc.vector.tensor_reduce(rmax[:], ml[:], axis=AxX, op=Alu.max)
                    ebias = ep.tile([128, 1], F32, tag="ebias")
                    nc.vector.tensor_scalar_mul(ebias[:], rmax[:], -1.0 / S)
                    pex = ep.tile([128, E], F32, tag="pex")
                    psm = ep.tile([128, 1], F32, tag="psm")
                    nc.scalar.activation(
                        pex[:], ml[:], Act.Exp, bias=ebias[:], scale=1.0 / S,
                        accum_out=psm[:],
                    )
                    prc = ep.tile([128, 1], F32, tag="prc")
                    nc.vector.reciprocal(prc[:], psm[:])
                    probs = ep.tile([128, E], F32, tag="probs")
                    nc.vector.tensor_scalar_mul(probs[:], pex[:], prc[:])

                    # per-sample softmax via per-s ACT exp with bias = -max_s
                    rmx = ep.tile([128, S], F32, tag="rmx")
                    nc.vector.tensor_reduce(rmx[:], lgd[:], axis=AxX, op=Alu.max)
                    nrmx = ep.tile([128, S], F32, tag="nrmx")
                    nc.vector.tensor_scalar_mul(nrmx[:], rmx[:], -1.0)
                    aex = ep.tile([128, S, E], F32, tag="aex")
                    asm = ep.tile([128, S], F32, tag="asm")
                    for s in range(S):
                        nc.scalar.activation(
                            aex[:, s, :], lgd[:, s, :], Act.Exp,
                            bias=nrmx[:, s : s + 1], accum_out=asm[:, s : s + 1],
                        )
                    arc = ep.tile([128, S, 1], F32, tag="arc")
                    nc.vector.reciprocal(arc[:], asm[:][:, :, None])
                    aprobs = ep.tile([128, S, E], F32, tag="aprobs")
                    nc.vector.tensor_mul(
                        aprobs[:], aex[:], arc[:].to_broadcast([128, S, E])
                    )

                    # two-pass std over samples (ddof=1), mean over experts
                    sump = ep.tile([128, E], F32, tag="sump")
                    nc.vector.tensor_reduce(
                        sump[:],
                        aprobs[:].rearrange("p s e -> p e s"),
                        axis=AxX,
                        op=Alu.add,
                    )
                    dev = ep.tile([128, S, E], F32, tag="dev")
                    nc.vector.scalar_tensor_tensor(
                        dev[:],
                        sump[:][:, None, :].to_broadcast([128, S, E]),
                        -1.0 / S,
                        aprobs[:],
                        op0=Alu.mult,
                        op1=Alu.add,
                    )
                    sq = ep.tile([128, S, E], F32, tag="sq")
                    nc.vector.tensor_mul(sq[:], dev[:], dev[:])
                    vars_ = ep.tile([128, E], F32, tag="vars")
                    nc.vector.tensor_reduce(
                        vars_[:], sq[:].rearrange("p s e -> p e s"), axis=AxX, op=Alu.add
                    )
                    stdt = ep.tile([128, E], F32, tag="stdt")
                    usum = ep.tile([128, 1], F32, tag="usum")
                    nc.scalar.activation(
                        stdt[:], vars_[:], Act.Sqrt, scale=1.0 / (S - 1),
                        accum_out=usum[:],
                    )
                    nc.vector.tensor_scalar_mul(up[:, t : t + 1], usum[:], 1.0 / E)

                    # top-k
                    mv = ep.tile([128, 8], F32, tag="mv")
                    nc.vector.max(out=mv[:], in_=probs[:])
                    mi = ep.tile([128, 8], U32, tag="mi")
                    nc.vector.max_index(out=mi[:], in_max=mv[:], in_values=probs[:])

                    mg = ep.tile([128, 1], F32, tag="mg")
                    nc.vector.tensor_scalar(
                        mg[:], up[:, t : t + 1], THR, None, op0=Alu.is_gt
                    )
                    mgi = ep.tile([128, 1], I32, tag="mgi")
                    nc.vector.tensor_scalar(
                        mgi[:], up[:, t : t + 1], THR, None, op0=Alu.is_gt
                    )
                    nc.vector.tensor_copy(pp[:, t, :], mv[:, :4])
                    nc.vector.tensor_scalar_mul(pp[:, t, 2:4], pp[:, t, 2:4], mg[:])
                    mi32 = ep.tile([128, 4], I32, tag="mi32")
                    nc.vector.tensor_copy(mi32[:], mi[:, :4])
                    nc.vector.tensor_copy(ip[:, t, :2], mi32[:, :2])
                    nc.vector.memset(ip[:, t, 2:4], -1)
                    nc.vector.copy_predicated(
                        ip[:, t, 2:4], mgi[:].to_broadcast([128, 2]), mi32[:, 2:4]
                    )
                    if j == 3:
                        t0 = t - 3
                        nc.sync.dma_start(
                            ip_d[:, t0 : t0 + 4, :], ip[:, t0 : t0 + 4, :]
                        )
                        nc.sync.dma_start(
                            pp_d[:, t0 : t0 + 4, :], pp[:, t0 : t0 + 4, :]
                        )
                        nc.sync.dma_start(up_d[:, t0 : t0 + 4], up[:, t0 : t0 + 4])

                return emit

            pending = []  # deferred per-tile epilogues from the previous block
            mm_q = []  # skewed L1 matmul emission

            def flush_mm():
                while mm_q:
                    mm_q.pop(0)()

            for blk in range(nblk):
                b0 = blk * 512
                h_ps = hps.tile([128, 512], F32, tag="hps")

                def make_mm(h_ps, xt_sb, kk, k):
                    def emit():
                        nc.tensor.matmul(
                            h_ps[:],
                            w1_sb[:, k, :],
                            xt_sb[:, kk, :],
                            start=(k == 0),
                            stop=(k == KCH - 1),
                        )

                    return emit

                def make_m1_piece(pair, b0):
                    def emit():
                        mp_ps = m1ps.tile([128, 2, 512], F32, tag="m1ps")
                        for si, s in enumerate(pair):
                            m1n = m1pool.tile([128, 4, 128], F32, tag="m1n")
                            nc.scalar.dma_start(
                                m1n[:],
                                m1_d[s, b0 : b0 + 512, :].rearrange(
                                    "(j p) h -> p j h", p=128
                                ),
                            )
                            for j in range(4):
                                nc.tensor.transpose(
                                    mp_ps[:, si, j * 128 : (j + 1) * 128],
                                    m1n[:, j, :],
                                    ident[:],
                                )
                        m1t = m1tpool.tile([128, 2, 512], F32, tag="m1t")
                        np_ = len(pair)
                        nc.vector.tensor_scalar(
                            m1t[:, :np_, :],
                            mp_ps[:, :np_, :],
                            P_DROP,
                            None,
                            op0=Alu.is_ge,
                        )
                        m1t_parts.append((m1t, np_))

                    return emit

                m1t_parts = []
                m1_q = [make_m1_piece(p, b0) for p in ((0, 1), (2, 3), (4,))]

                if blk == 0:
                    param_prep()
                for kc in range(4):
                    for dk2 in range(4):  # two k-chunks per 2-bank psum tile
                        d0 = kc * 1024 + dk2 * 256
                        xc = xpool.tile([128, 4, 256], F32, tag="xc")
                        nc.sync.dma_start(
                            xc[:],
                            x_d[b0 : b0 + 512, d0 : d0 + 256].rearrange(
                                "(j p) d -> p j d", p=128
                            ),
                        )
                        xt_ps = tps.tile([128, 2, 512], F32, tag="tps")
                        for kk in range(2):
                            for j in range(4):
                                nc.tensor.transpose(
                                    xt_ps[:, kk, j * 128 : (j + 1) * 128],
                                    xc[:, j, kk * 128 : (kk + 1) * 128],
                                    ident[:],
                                )
                            if mm_q:
                                mm_q.pop(0)()
                        xt_sb = xtpool.tile([128, 2, 512], F32, tag="xt")
                        psum_copy(xt_sb[:], xt_ps[:])
                        k0 = kc * 8 + dk2 * 2
                        mm_q.append(make_mm(h_ps, xt_sb, 0, k0))
                        mm_q.append(make_mm(h_ps, xt_sb, 1, k0 + 1))
                        if dk2 == 1 and m1_q:
                            m1_q.pop(0)()
                        elif dk2 == 3 and pending:
                            pending.pop(0)()
                flush_mm()
                while m1_q:
                    m1_q.pop(0)()
                while pending:
                    pending.pop(0)()

                h_sb = hpool.tile([128, 512], F32, tag="hsb")
                nc.scalar.activation(
                    h_sb[:], h_ps[:], Act.Relu, bias=b1s[:], scale=SCALE
                )

                hm = []
                for si_s in m1t_parts:
                    m1t, np_ = si_s
                    for si in range(np_):
                        s = len(hm)
                        hms = hmpool.tile([128, 512], F32, tag=f"hm{s}")
                        nc.vector.tensor_mul(hms[:], h_sb[:], m1t[:, si, :])
                        hm.append(hms)

                pending = [make_epilogue(blk * 4 + j, hm) for j in range(4)]

            while pending:
                pending.pop(0)()

    nc.compile()
    if not nc.is_finalized():
        nc.finalize()
    return nc


_NC_CACHE = {}


def _get_nc(b_loc):
    if b_loc not in _NC_CACHE:
        _NC_CACHE[b_loc] = build(b_loc)
    return _NC_CACHE[b_loc]


def _unpack(res_c, b_loc):
    nt = b_loc // 128
    idx = np.ascontiguousarray(
        res_c["idx_pack"].transpose(1, 0, 2).reshape(b_loc, 4)
    ).astype(np.int32)
    prb = np.ascontiguousarray(
        res_c["prob_pack"].transpose(1, 0, 2).reshape(b_loc, 4)
    ).astype(np.float32)
    unc = np.ascontiguousarray(res_c["unc_pack"].transpose(1, 0).reshape(b_loc)).astype(
        np.float32
    )
    return idx, prb, unc


def run_sharded(x, W1, b1, W2, b2, mask1_u, mask2_u, n_cores=N_CORES, **kw):
    x = np.asarray(x, np.float32)
    W1 = np.asarray(W1, np.float32)
    b1 = np.asarray(b1, np.float32)
    W2 = np.asarray(W2, np.float32)
    b2 = np.asarray(b2, np.float32)
    mask1_u = np.asarray(mask1_u, np.float32)
    mask2_u = np.asarray(mask2_u, np.float32)

    B = x.shape[0]
    b_loc = B // n_cores
    nc = _get_nc(b_loc)
    in_maps = []
    for c in range(n_cores):
        sl = slice(c * b_loc, (c + 1) * b_loc)
        in_maps.append(
            {
                "xs": np.ascontiguousarray(x[sl]),
                "m1s": np.ascontiguousarray(mask1_u[:, sl, :]),
                "m2s": np.ascontiguousarray(mask2_u[:, sl, :]),
                "w1": W1,
                "b1": b1,
                "w2": W2,
                "b2": b2,
            }
        )
    res = run_bass_kernel_spmd(nc, in_maps, core_ids=list(range(n_cores)), **kw)
    outs = [_unpack(r, b_loc) for r in res.results]
    out_idx = np.concatenate([o[0] for o in outs], axis=0)
    out_probs = np.concatenate([o[1] for o in outs], axis=0)
    unc = np.concatenate([o[2] for o in outs], axis=0)
    return (out_idx, out_probs, unc), res


def kernel(x, W1, b1, W2, b2, mask1_u, mask2_u):
    (out_idx, out_probs, unc), _ = run_sharded(x, W1, b1, W2, b2, mask1_u, mask2_u)
    return out_idx, out_probs, unc
